# revision 1
# baseline (speedup 1.0000x reference)
"""Bilaplacian of a 2-layer tanh MLP on 8 TRN2 NeuronCores.

The reference computes sum_{i,j} d^4 f / dx_i^2 dx_j^2 at a point x via
6112 fourth directional derivatives (Taylor-mode) of
f(z) = W3 tanh(W2 tanh(W1 z + b1) + b2) + b3 and polarization weights.
Because the first layer is affine in the direction v and all tanh
derivatives are evaluated at the shared point x, the weighted direction
sum collapses in closed form to Gram-matrix contractions (no directions
on device at all): using
  sum_v w_v (a.v)(b.v)(c.v)(e.v) = ((a.b)(c.e)+(a.c)(b.e)+(a.e)(b.c))/3
(validated against the reference to 7e-15 in float64), the result is
24 * W3 @ G4 with per-row terms built from K = W1 W1^T, B1K = B1 K,
CK2 = B2 (K*K) and row-sums of Hadamard products, where
B_k = W2 diag(t_k/k!) with t_k the k-th tanh derivative at
u0 = W1 x + b1 and d_k the tanh derivatives at a0 = W2 tanh(u0) + b2.

Output rows are sharded 32/core over all 8 cores (no collectives; the
host sums the 8 partial W3-dot products). fp32 throughout: the
direction-sum cancellation (~650x) makes bf16/float32r matmuls fail
(measured). B1K = (B1 W1) W1^T computed as
  BW = W1^T B1^T   (two accumulating matmuls, free=32)
  B1K = BW^T @ W1^T (one matmul, contract 64, free 256)
so the K PSUM->SBUF copies vanish (K*K is squared straight from PSUM on
ACT) and total PE streaming drops from 1536 to 1344 free-rows.

Inputs packed host-side:
  w1t  (64, 256)   W1^T
  btp  (128, 256)  [B1T0|B1T1|B2T0|B2T1|W1r0|W1r1]
  rowp (32, 1024)  [b1row|b3rd2|b2d2|b2d3]   (final combine on host)
"""

import numpy as np

D = 64
H = 256
N_CORES = 8
R = H // N_CORES
N_WARM = 3  # PE HAM warmup matmuls (fills the DMA-wait window)

_CACHE = {}


def _build():
    if "nc" in _CACHE:
        return _CACHE["nc"]

    import concourse.bass as bass  # noqa: F401
    import concourse.tile as tile
    from concourse import bacc, mybir

    f32 = mybir.dt.float32
    bf16 = mybir.dt.bfloat16
    mult = mybir.AluOpType.mult
    add = mybir.AluOpType.add
    X = mybir.AxisListType.X

    nc = bacc.Bacc(
        "TRN2",
        target_bir_lowering=False,
        debug=False,
        enable_asserts=False,
        num_devices=N_CORES,
    )

    w1t_d = nc.dram_tensor("w1t", [D, H], f32, kind="ExternalInput").ap()
    btp_d = nc.dram_tensor("btp", [128, 4 * R + 2 * D], f32,
                           kind="ExternalInput").ap()
    rowp_d = nc.dram_tensor("rowp", [R, 4 * H], f32, kind="ExternalInput").ap()
    out_d = nc.dram_tensor("g4", [R, 3], f32, kind="ExternalOutput").ap()
    warm_d = nc.dram_tensor("warm", [1, 1], f32, kind="ExternalOutput").ap()

    with tile.TileContext(nc) as tc:
        with (
            tc.tile_pool(name="consts", bufs=1) as consts,
            tc.tile_pool(name="ksb", bufs=1) as ksb,
            tc.tile_pool(name="scr", bufs=1) as scr,
            tc.tile_pool(name="small", bufs=1) as small,
            tc.tile_pool(name="kpsum", bufs=1, space="PSUM") as kpsum,
            tc.tile_pool(name="bpsum", bufs=1, space="PSUM") as bpsum,
        ):
            # ---- PE warmup (real-HW HAM clock-gate ramp; model-neutral) ----
            warm_in = consts.tile([128, 512], bf16, tag="warm_in")
            nc.vector.memset(warm_in[:], 0)
            wpsum = kpsum.tile([128, 512], f32, tag="wpsum")
            for _ in range(N_WARM):
                nc.tensor.matmul(wpsum[:], warm_in[:, 0:128], warm_in[:],
                                 start=True, stop=True)

            # ---- 3 input loads on 3 different sequencers ----
            w1t_s = consts.tile([D, H], f32, tag="w1t")
            nc.sync.dma_start(w1t_s[:], w1t_d[:])
            btp_s = consts.tile([128, 4 * R + 2 * D], f32, tag="btp")
            nc.scalar.dma_start(btp_s[:], btp_d[:])
            rowp_s = consts.tile([R, 4 * H], f32, tag="rowp")
            nc.gpsimd.dma_start(rowp_s[:], rowp_d[:])

            b1t = [btp_s[:, 32 * h:32 * (h + 1)] for h in range(2)]
            b2t = [btp_s[:, 64 + 32 * h:64 + 32 * (h + 1)] for h in range(2)]
            w1r = [btp_s[:, 128 + 64 * h:128 + 64 * (h + 1)] for h in range(2)]
            b1row = rowp_s[:, 0:H]
            b3rd2 = rowp_s[:, H:2 * H]
            b2d2 = rowp_s[:, 2 * H:3 * H]
            b2d3 = rowp_s[:, 3 * H:4 * H]

            # ---- PE: K = W1 W1^T; BW = W1^T B1^T; B1K = BW^T W1^T; CK2 ----
            kp0 = kpsum.tile([128, H], f32, tag="kp0")
            nc.tensor.matmul(kp0[:], w1t_s[:, 0:128], w1t_s[:],
                             start=True, stop=True)
            bw = bpsum.tile([D, R], f32, tag="bw")
            nc.tensor.matmul(bw[:], w1r[0], b1t[0], start=True, stop=False)
            nc.tensor.matmul(bw[:], w1r[1], b1t[1], start=False, stop=True)
            kp1 = kpsum.tile([128, H], f32, tag="kp1")
            nc.tensor.matmul(kp1[:], w1t_s[:, 128:256], w1t_s[:],
                             start=True, stop=True)
            kp = [kp0, kp1]

            # ACT: BW copy between the kk squares
            bws = ksb.tile([D, R], f32, tag="bws")
            nc.scalar.copy(bws[:], bw[:])
            kk0 = ksb.tile([128, H], f32, tag="kk0")
            nc.scalar.square(kk0[:], kp[0][:])
            kk1 = ksb.tile([128, H], f32, tag="kk1")
            nc.scalar.square(kk1[:], kp[1][:])

            b1k = bpsum.tile([R, H], f32, tag="b1k")
            nc.tensor.matmul(b1k[:], bws[:], w1t_s[:], start=True, stop=True)
            ck2 = bpsum.tile([R, H], f32, tag="ck2")
            nc.tensor.matmul(ck2[:], b2t[0], kk0[:], start=True, stop=False)
            nc.tensor.matmul(ck2[:], b2t[1], kk1[:], start=False, stop=True)

            # ---- Hadamard + row reductions ----
            # [sc1|sc4] share one (R,512) scratch -> one reduce for t1+t3b
            sca = scr.tile([R, 2 * H], f32, tag="sca")
            sc2 = scr.tile([R, H], f32, tag="sc2")
            sc3 = scr.tile([R, H], f32, tag="sc3")
            q2m = scr.tile([R, H], f32, tag="q2m")
            outs = small.tile([R, 3], f32, tag="outs")

            nc.vector.tensor_mul(sca[:, 0:H], b1k[:], b3rd2)
            nc.vector.tensor_mul(sc2[:], b1k[:], b1row)
            nc.vector.reduce_sum(outs[:, 2:3], sc2[:], axis=X)
            nc.scalar.square(q2m[:], b1k[:])
            nc.vector.tensor_mul(sca[:, H:2 * H], q2m[:], b2d3)
            nc.vector.reduce_sum(outs[:, 0:1], sca[:], axis=X)
            nc.vector.tensor_mul(sc3[:], ck2[:], b2d2)
            # t2b reduce on ACT (Copy + accum) — DVE is saturated here
            dum = scr.tile([R, H], f32, tag="dum")
            nc.scalar.activation(
                dum[:], sc3[:], mybir.ActivationFunctionType.Copy,
                accum_out=outs[:, 1:2])

            # final combine happens on host: g4 = t13 + t2b + s1*c2+ht + d4h*s1^2
            nc.sync.dma_start(out_d[:], outs[:])

            # warm PSUM must stay live (DCE) — copy 4B out at the end on ACT
            warm_out = small.tile([1, 1], f32, tag="warm_out")
            nc.scalar.copy(warm_out[:], wpsum[0:1, 0:1])
            nc.scalar.dma_start(warm_d[:], warm_out[:])

    nc.compile()
    _CACHE["nc"] = nc
    return nc


def make_in_maps(x, W1, b1, W2, b2, W3, b3):
    u0 = W1 @ x + b1
    y = np.tanh(u0)
    p = 1.0 - y * y
    e1 = p
    e2 = -y * p
    e3 = p * (y * y - np.float32(1.0 / 3.0))
    e4 = y * p * (2.0 - 3.0 * y * y) / 3.0

    a0 = W2 @ y + b2
    s = np.tanh(a0)
    q = 1.0 - s * s
    d1 = q
    d2 = -2.0 * s * q
    d3h = q * (3.0 * s * s - 1.0)
    d4h = s * q * (2.0 - 3.0 * s * s) / 3.0

    r = np.sum(W1 * W1, axis=1)
    B2r = W2 @ (e2 * r)
    Ta4 = W2 @ (e4 * r * r)
    ht = d1 * Ta4 + (d2 / 6.0) * B2r * B2r
    c2 = (d3h / 3.0) * B2r

    W1T = np.ascontiguousarray(W1.T)
    W2T = np.ascontiguousarray(W2.T)
    B1T = W2T * e1[:, None]
    B2T = W2T * e2[:, None]
    B1row = W2 * e1[None, :]
    B3r_row = W2 * (e3 * r)[None, :]
    B2row = W2 * e2[None, :]

    in_maps = []
    for c in range(N_CORES):
        bs = slice(c * R, (c + 1) * R)
        btp = np.concatenate(
            [B1T[0:128, bs], B1T[128:256, bs], B2T[0:128, bs], B2T[128:256, bs],
             W1[0:128, :], W1[128:256, :]], axis=1)
        rowp = np.concatenate(
            [B1row[bs],
             B3r_row[bs] * d2[bs, None],
             B2row[bs] * (d2[bs, None] / 3.0),
             B2row[bs] * (2.0 * d3h[bs, None] / 3.0)],
            axis=1)
        in_maps.append({
            "w1t": W1T,
            "btp": np.ascontiguousarray(btp),
            "rowp": np.ascontiguousarray(rowp),
        })
    return in_maps


def kernel(x, W1, b1, W2, b2, W3, b3):
    from concourse import bass_utils

    args = [np.asarray(a, np.float32) for a in (x, W1, b1, W2, b2, W3, b3)]
    x, W1, b1, W2, b2, W3, b3 = args
    in_maps = make_in_maps(*args)
    nc = _build()
    res = bass_utils.run_bass_kernel_spmd(
        nc, in_maps, core_ids=list(range(N_CORES)))
    parts = np.concatenate([res.results[c]["g4"] for c in range(N_CORES)], 0)
    t13, t2b, s1 = parts[:, 0], parts[:, 1], parts[:, 2]
    # host combine (O(H)): same scalars as make_in_maps
    u0 = W1 @ x + b1
    y = np.tanh(u0); p = 1.0 - y * y
    e2 = -y * p; e4 = y * p * (2.0 - 3.0 * y * y) / 3.0
    a0 = W2 @ y + b2
    s = np.tanh(a0); q = 1.0 - s * s
    d1 = q; d2 = -2.0 * s * q
    d3h = q * (3.0 * s * s - 1.0); d4h = s * q * (2.0 - 3.0 * s * s) / 3.0
    r = np.sum(W1 * W1, axis=1)
    B2r = W2 @ (e2 * r)
    Ta4 = W2 @ (e4 * r * r)
    ht = d1 * Ta4 + (d2 / 6.0) * B2r * B2r
    c2 = (d3h / 3.0) * B2r
    g4 = t13 + t2b + s1 * c2 + ht + d4h * s1 * s1
    out = 24.0 * np.float32(W3[0] @ g4)
    return np.array([out], dtype=np.float32)

